# revision 5
# baseline (speedup 1.0000x reference)
"""Causal self-attention (B=4, T=2048, C=1024, H=16) on 8 TRN2 NeuronCores.

Sharding: 8 cores = 4 batches x 2 head-groups (Megatron tensor-parallel over
heads + data-parallel over batch). Each core computes, for its batch b and its
8 heads:
  stage 1: qkv projection (column-parallel slice of w_qkv)
           -> qT, kT stored [head_dim on partitions, T on free] (pair-packed:
              even head dims on partitions 0-63, odd head on 64-127)
           -> V stored [T on partitions, head-major dims on free], with an
              appended ones-column per head (computes softmax sums for free)
  stage 2: causal attention computed TRANSPOSED (S^T = K^T-tiles x Q-tiles,
           keys on PSUM partitions, queries on free dim):
           - K=64 contractions pair-packed via tile_position row groups
           - causal triangle applied by accumulating a -1e5 triangular mask
             into PSUM via a bf16 identity matmul (only diagonal j-tiles)
           - exp on the Scalar engine straight out of PSUM (no max-subtraction:
             logits are O(1) by construction, 1/sqrt(hd) folded into w_q)
           - PV matmul with ones-augmented V gives unnormalized y^T plus the
             softmax sums in PSUM row 64; normalize with reciprocal +
             partition_broadcast
  stage 3: output projection partial (row-parallel slice of w_proj)
Host: shards inputs, sums the two partial outputs per batch, adds b_proj.

All heavy matmuls run as float32r (TF32-like, full PE rate at N>=256; inputs
are fp32 bits reinterpreted via bitcast).
"""

import sys

if "/opt/trn_rl_repo" not in sys.path:
    sys.path.insert(0, "/opt/trn_rl_repo")

from contextlib import ExitStack

import numpy as np
import ml_dtypes

import concourse.bass as bass
import concourse.tile as tile
from concourse import bacc, mybir
from concourse.bass_utils import run_bass_kernel_spmd

F32 = mybir.dt.float32
F32R = mybir.dt.float32r
BF16 = mybir.dt.bfloat16
AF = mybir.ActivationFunctionType

B, T, C = 4, 2048, 1024
H, HD = 16, 64
NHL = 8          # heads per core (local)
NPAIR = 4        # head pairs per core
P = 128
TQ = 512         # query tile (free dim)
TJ = 128         # key tile (partitions)
NIT = T // TQ    # 4 query tiles
NTS = T // P     # 16 token sub-tiles
NCT = C // P     # 8 contraction tiles over C
NEG = -100000.0  # additive causal mask value


def build_kernel(trace_label=None):
    nc = bacc.Bacc("TRN2", target_bir_lowering=False)

    xt = nc.declare_dram_parameter("xt", [C, T], F32R, isOutput=False)
    wqk = nc.declare_dram_parameter("wqk", [P, NCT, 1024], F32R, isOutput=False)
    bqk = nc.declare_dram_parameter("bqk", [P, 8], F32, isOutput=False)
    wv = nc.declare_dram_parameter("wv", [P, NCT, 512], F32R, isOutput=False)
    bv = nc.declare_dram_parameter("bv", [1, 512], F32, isOutput=False)
    wp = nc.declare_dram_parameter("wp", [P, NPAIR, 1024], F32R, isOutput=False)
    tri = nc.declare_dram_parameter("tri", [P, P], BF16, isOutput=False)
    idn = nc.declare_dram_parameter("idn", [P, P], BF16, isOutput=False)
    out = nc.declare_dram_parameter("out", [T, C], F32, isOutput=True)

    with tile.TileContext(nc) as tc, ExitStack() as ctx:
        persist = ctx.enter_context(tc.tile_pool(name="persist", bufs=1))

        q_sb = persist.tile([P, NPAIR, T], F32R)
        k_sb = persist.tile([P, NPAIR, T], F32R)
        v_sb = persist.tile([P, NTS, NHL, HD + 1], F32R)
        bqk_sb = persist.tile([P, 8], F32)
        bv_sb = persist.tile([P, 512], F32)
        tri_sb = persist.tile([P, P], BF16)
        idn_sb = persist.tile([P, P], BF16)
        wp_sb = persist.tile([P, NPAIR, 1024], F32R)

        nc.sync.dma_start(bqk_sb, bqk[:])
        nc.sync.dma_start(tri_sb, tri[:])
        nc.sync.dma_start(idn_sb, idn[:])
        nc.sync.dma_start(wp_sb, wp[:])
        # materialize bias-v broadcast across partitions
        nc.sync.dma_start(bv_sb[0:1, :], bv[:])
        nc.gpsimd.partition_broadcast(bv_sb[:, :], bv_sb[0:1, :])
        # ones columns of the augmented V
        nc.vector.memset(v_sb[:, :, :, HD : HD + 1].bitcast(F32), 1.0)

        # ---------------- stage 1: qkv projection ----------------
        with (
            tc.tile_pool(name="s1w", bufs=1) as s1w,
            tc.tile_pool(name="s1x", bufs=12) as s1x,
            tc.tile_pool(name="s1ps", bufs=3, space="PSUM") as s1ps,
        ):
            wqk_sb = s1w.tile([P, NCT, 1024], F32R)
            wv_sb = s1w.tile([P, NCT, 512], F32R)
            nc.sync.dma_start(wqk_sb, wqk[:])
            nc.sync.dma_start(wv_sb, wv[:])

            for t in range(NIT):
                t0 = t * TQ
                xc = []
                for c in range(NCT):
                    xi = s1x.tile([P, TQ], F32R, tag="xc")
                    nc.sync.dma_start(xi, xt[c * P : (c + 1) * P, t0 : t0 + TQ])
                    xc.append(xi)
                # q (m 0-3) and k (m 4-7) blocks: out [f-part, t-free]
                for m in range(8):
                    ps = s1ps.tile([P, TQ], F32, tag="ps")
                    for c in range(NCT):
                        nc.tensor.matmul(
                            ps,
                            wqk_sb[:, c, m * P : (m + 1) * P],
                            xc[c],
                            start=(c == 0),
                            stop=(c == NCT - 1),
                        )
                    dst = q_sb if m < 4 else k_sb
                    nc.scalar.activation(
                        dst[:, m % 4, t0 : t0 + TQ], ps, AF.Identity,
                        bias=bqk_sb[:, m : m + 1],
                    )
                # v blocks: out [t-part, f-free(head-major)]
                for s in range(TQ // P):
                    ps = s1ps.tile([P, 512], F32, tag="ps")
                    for c in range(NCT):
                        nc.tensor.matmul(
                            ps,
                            xc[c][:, s * P : (s + 1) * P],
                            wv_sb[:, c, :],
                            start=(c == 0),
                            stop=(c == NCT - 1),
                        )
                    tsub = t * (TQ // P) + s
                    nc.vector.tensor_tensor(
                        v_sb[:, tsub, :, 0:HD],
                        ps.rearrange("p (h d) -> p h d", h=NHL),
                        bv_sb.rearrange("p (h d) -> p h d", h=NHL),
                        mybir.AluOpType.add,
                    )

        # ---------------- stage 2: causal attention ----------------
        with (
            tc.tile_pool(name="s2att", bufs=6) as s2att,
            tc.tile_pool(name="s2n", bufs=3) as s2n,
            tc.tile_pool(name="qkps", bufs=4, space="PSUM") as qkps,
            tc.tile_pool(name="pvps", bufs=4, space="PSUM") as pvps,
        ):
            for a in range(NPAIR):
                for it in range(NIT):
                    i0 = it * TQ
                    njt = (i0 + TQ) // TJ
                    pv = [
                        pvps.tile([P, TQ], F32, tag="pv", name=f"pv0_{a}_{it}"),
                        pvps.tile([P, TQ], F32, tag="pv", name=f"pv1_{a}_{it}"),
                    ]
                    for jt in range(njt):
                        j0 = jt * TJ
                        d = j0 - i0
                        istart = max(d, 0)
                        nn = TQ - istart
                        for e in (0, 1):
                            qk = qkps.tile([P, TQ], F32, tag="qk")
                            nc.tensor.matmul(
                                qk[:, istart:TQ],
                                k_sb[64 * e : 64 * e + 64, a, j0 : j0 + TJ],
                                q_sb[64 * e : 64 * e + 64, a, i0 + istart : i0 + TQ],
                                start=True,
                                stop=(d < 0),
                                tile_position=(64 * e, 0),
                            )
                            if d >= 0:
                                nc.tensor.matmul(
                                    qk[:, istart : istart + TJ],
                                    idn_sb,
                                    tri_sb,
                                    start=False,
                                    stop=True,
                                    tile_position=(0, 0),
                                )
                            att = s2att.tile([P, TQ], F32R, tag="att")
                            nc.scalar.activation(att[:, 0:nn], qk[:, istart:TQ], AF.Exp)
                            nc.tensor.matmul(
                                pv[e][0 : HD + 1, istart:TQ],
                                v_sb[:, jt, 2 * a + e, :],
                                att[:, 0:nn],
                                start=(jt == 0),
                                stop=(jt == njt - 1),
                            )
                    for e in (0, 1):
                        rt = s2n.tile([P, TQ], F32, tag="rt")
                        nc.vector.reciprocal(rt[HD : HD + 1, :], pv[e][HD : HD + 1, :])
                        rb = s2n.tile([P, TQ], F32, tag="rb")
                        nc.sync.dma_start(rb[0:1, :], rt[HD : HD + 1, :])
                        nc.gpsimd.partition_broadcast(rb[0:HD, :], rb[0:1, :])
                        if e == 0:
                            nc.vector.tensor_mul(
                                q_sb[0:HD, a, i0 : i0 + TQ], pv[e][0:HD, :], rb[0:HD, :]
                            )
                        else:
                            yt = s2n.tile([P, TQ], F32R, tag="yt")
                            nc.vector.tensor_mul(yt[0:HD, :], pv[e][0:HD, :], rb[0:HD, :])
                            nc.sync.dma_start(
                                q_sb[64:128, a, i0 : i0 + TQ], yt[0:HD, :]
                            )

        # ---------------- stage 3: output projection ----------------
        # normalized y^T was written into q_sb (reuse; q no longer needed)
        with (
            tc.tile_pool(name="s3o", bufs=4) as s3o,
            tc.tile_pool(name="s3ps", bufs=4, space="PSUM") as s3ps,
        ):
            for tt in range(NTS):
                for ot in range(2):
                    ps = s3ps.tile([P, 512], F32, tag="ops")
                    for a in range(NPAIR):
                        nc.tensor.matmul(
                            ps,
                            q_sb[:, a, tt * P : (tt + 1) * P],
                            wp_sb[:, a, ot * 512 : (ot + 1) * 512],
                            start=(a == 0),
                            stop=(a == NPAIR - 1),
                        )
                    ot_sb = s3o.tile([P, 512], F32, tag="osb")
                    nc.vector.tensor_copy(ot_sb, ps)
                    nc.sync.dma_start(
                        out[tt * P : (tt + 1) * P, ot * 512 : (ot + 1) * 512], ot_sb
                    )

    nc.compile()
    return nc


_NC_CACHE = None


def _get_nc():
    global _NC_CACHE
    if _NC_CACHE is None:
        _NC_CACHE = build_kernel()
    return _NC_CACHE


def _shard_inputs(x, w_qkv, b_qkv, w_proj):
    """Build the 8 per-core input maps. Core id = 2*batch + head_group."""
    tri_np = np.where(
        np.arange(P)[None, :] >= np.arange(P)[:, None], 0.0, NEG
    ).astype(ml_dtypes.bfloat16)
    idn_np = np.eye(P, dtype=ml_dtypes.bfloat16)

    in_maps = []
    for b in range(B):
        xt = np.ascontiguousarray(x[b].T)  # [C, T]
        for g in range(2):
            s = slice(g * 512, (g + 1) * 512)
            wqk_full = np.concatenate(
                [w_qkv[0:1024][s] / 8.0, w_qkv[1024:2048][s]], axis=0
            )  # [1024 f, 1024 c]
            wqk_arr = np.ascontiguousarray(
                wqk_full.T.reshape(NCT, P, 1024).transpose(1, 0, 2)
            )
            bqk_full = np.concatenate([b_qkv[0:1024][s] / 8.0, b_qkv[1024:2048][s]])
            bqk_arr = np.ascontiguousarray(bqk_full.reshape(8, P).T)
            wv_rows = w_qkv[2048:3072][s]  # [512 f, 1024 c]
            wv_arr = np.ascontiguousarray(
                wv_rows.T.reshape(NCT, P, 512).transpose(1, 0, 2)
            )
            bv_arr = np.ascontiguousarray(b_qkv[2048:3072][s][None, :])
            wp_rhs = w_proj[:, s].T  # [512 hd, 1024 o]
            wp_arr = np.ascontiguousarray(
                wp_rhs.reshape(NPAIR, P, 1024).transpose(1, 0, 2)
            )
            in_maps.append(
                {
                    "xt": xt,
                    "wqk": wqk_arr.astype(np.float32),
                    "bqk": bqk_arr.astype(np.float32),
                    "wv": wv_arr.astype(np.float32),
                    "bv": bv_arr.astype(np.float32),
                    "wp": wp_arr.astype(np.float32),
                    "tri": tri_np,
                    "idn": idn_np,
                }
            )
    return in_maps


def kernel(x, w_qkv, b_qkv, w_proj, b_proj, _trace=False, _trace_kwargs=None):
    x = np.asarray(x, dtype=np.float32)
    w_qkv = np.asarray(w_qkv, dtype=np.float32)
    b_qkv = np.asarray(b_qkv, dtype=np.float32)
    w_proj = np.asarray(w_proj, dtype=np.float32)
    b_proj = np.asarray(b_proj, dtype=np.float32)

    nc = _get_nc()
    in_maps = _shard_inputs(x, w_qkv, b_qkv, w_proj)
    res = run_bass_kernel_spmd(
        nc, in_maps, core_ids=list(range(8)), trace=_trace,
        **(_trace_kwargs or {}),
    )
    out = np.empty((B, T, C), np.float32)
    for b in range(B):
        out[b] = res.results[2 * b]["out"] + res.results[2 * b + 1]["out"] + b_proj
    if _trace:
        return out, res
    return out


# revision 16
# speedup vs baseline: 50.9386x; 50.9386x over previous
"""Causal self-attention (B=4, T=2048, C=1024, H=16) on 8 TRN2 NeuronCores.

Sharding: 8 cores = 4 batches x 2 head-groups (Megatron tensor-parallel over
heads + data-parallel over batch). Each core computes, for its batch b and its
8 heads:
  stage 1: qkv projection (column-parallel slice of w_qkv)
           -> qT, kT stored [head_dim on partitions, T on free] (pair-packed:
              even head dims on partitions 0-63, odd head on 64-127)
           -> V stored [T on partitions, head-major dims on free], with an
              appended ones-column per head (computes softmax sums for free)
  stage 2: causal attention computed TRANSPOSED (S^T = K^T-tiles x Q-tiles,
           keys on PSUM partitions, queries on free dim):
           - K=64 contractions pair-packed via tile_position row groups
           - causal triangle applied by accumulating a -1e5 triangular mask
             into PSUM via a bf16 identity matmul (only diagonal j-tiles)
           - exp on the Scalar engine straight out of PSUM (no max-subtraction:
             logits are O(1) by construction, 1/sqrt(hd) folded into w_q)
           - PV matmul with ones-augmented V gives unnormalized y^T plus the
             softmax sums in PSUM row 64; normalize with reciprocal +
             partition_broadcast
  stage 3: output projection partial (row-parallel slice of w_proj)
Host: shards inputs, sums the two partial outputs per batch, adds b_proj.

All heavy matmuls run as float32r (TF32-like, full PE rate at N>=256; inputs
are fp32 bits reinterpreted via bitcast).
"""

import sys

if "/opt/trn_rl_repo" not in sys.path:
    sys.path.insert(0, "/opt/trn_rl_repo")

from contextlib import ExitStack

import numpy as np
import ml_dtypes

import concourse.bass as bass
import concourse.tile as tile
from concourse import bacc, mybir
from concourse.bass_utils import run_bass_kernel_spmd

F32 = mybir.dt.float32
F32R = mybir.dt.float32r
BF16 = mybir.dt.bfloat16
AF = mybir.ActivationFunctionType

B, T, C = 4, 2048, 1024
H, HD = 16, 64
NHL = 8          # heads per core (local)
NPAIR = 4        # head pairs per core
P = 128
TQ = 512         # query tile (free dim)
TJ = 128         # key tile (partitions)
NIT = T // TQ    # 4 query tiles
NTS = T // P     # 16 token sub-tiles
NCT = C // P     # 8 contraction tiles over C
NEG = -100000.0  # additive causal mask value


def build_kernel(trace_label=None):
    nc = bacc.Bacc("TRN2", target_bir_lowering=False)

    xt = nc.declare_dram_parameter("xt", [C, T], F32R, isOutput=False)
    wqk = nc.declare_dram_parameter("wqk", [P, NCT, 1024], F32R, isOutput=False)
    bqk = nc.declare_dram_parameter("bqk", [P, 8], F32, isOutput=False)
    wv = nc.declare_dram_parameter("wv", [P, NCT, 512], F32R, isOutput=False)
    bv = nc.declare_dram_parameter("bv", [1, 512], F32, isOutput=False)
    wp = nc.declare_dram_parameter("wp", [P, NPAIR, 1024], F32R, isOutput=False)
    tri = nc.declare_dram_parameter("tri", [P, P], BF16, isOutput=False)
    idn = nc.declare_dram_parameter("idn", [P, P], BF16, isOutput=False)
    out = nc.declare_dram_parameter("out", [T, C], F32, isOutput=True)

    with tile.TileContext(nc) as tc, ExitStack() as ctx:
        persist = ctx.enter_context(tc.tile_pool(name="persist", bufs=1))

        q_sb = persist.tile([P, NPAIR, T], F32R)
        k_sb = persist.tile([P, NPAIR, T], F32R)
        v_sb = persist.tile([P, NTS, NHL, HD + 1], F32R)
        bqk_sb = persist.tile([P, 8], F32)
        bv_sb = persist.tile([P, 512], F32)
        tri_sb = persist.tile([P, P], BF16)
        idn_sb = persist.tile([P, P], BF16)
        wp_sb = persist.tile([P, NPAIR, 1024], F32R)

        nc.sync.dma_start(bqk_sb, bqk[:])
        nc.sync.dma_start(tri_sb, tri[:])
        nc.sync.dma_start(idn_sb, idn[:])
        # materialize bias-v broadcast across partitions
        nc.sync.dma_start(bv_sb[0:1, :], bv[:])
        nc.gpsimd.partition_broadcast(bv_sb[:, :], bv_sb[0:1, :])
        # ones columns of the augmented V
        nc.vector.memset(v_sb[:, :, :, HD : HD + 1].bitcast(F32), 1.0)

        # ---------------- stage 1: qkv projection ----------------
        with (
            tc.tile_pool(name="s1w", bufs=1) as s1w,
            tc.tile_pool(name="s1x", bufs=12) as s1x,
            tc.tile_pool(name="s1ps", bufs=3, space="PSUM") as s1ps,
        ):
            wqk_sb = s1w.tile([P, NCT, 1024], F32R)
            wv_sb = s1w.tile([P, NCT, 512], F32R)

            for t in range(NIT):
                t0 = t * TQ
                xc = []
                for c in range(NCT):
                    xi = s1x.tile([P, TQ], F32R, tag="xc")
                    nc.sync.dma_start(xi, xt[c * P : (c + 1) * P, t0 : t0 + TQ])
                    xc.append(xi)
                    if t == 0:
                        # stream weight chunks interleaved with the first x
                        # tiles so the c-accumulation paces with DMA arrival
                        # instead of stalling ~25us on monolithic loads
                        nc.sync.dma_start(wqk_sb[:, c, :], wqk[:, c, :])
                        nc.sync.dma_start(wv_sb[:, c, :], wv[:, c, :])
                # q (m 0-3) and k (m 4-7) blocks: out [f-part, t-free]
                for m in range(8):
                    ps = s1ps.tile([P, TQ], F32, tag="ps")
                    for c in range(NCT):
                        nc.tensor.matmul(
                            ps,
                            wqk_sb[:, c, m * P : (m + 1) * P],
                            xc[c],
                            start=(c == 0),
                            stop=(c == NCT - 1),
                        )
                    dst = q_sb if m < 4 else k_sb
                    nc.vector.tensor_scalar_add(
                        dst[:, m % 4, t0 : t0 + TQ], ps, bqk_sb[:, m : m + 1]
                    )
                # v blocks: out [t-part, f-free(head-major)]
                for s in range(TQ // P):
                    ps = s1ps.tile([P, 512], F32, tag="ps")
                    for c in range(NCT):
                        nc.tensor.matmul(
                            ps,
                            xc[c][:, s * P : (s + 1) * P],
                            wv_sb[:, c, :],
                            start=(c == 0),
                            stop=(c == NCT - 1),
                        )
                    tsub = t * (TQ // P) + s
                    nc.vector.tensor_tensor(
                        v_sb[:, tsub, :, 0:HD],
                        ps.rearrange("p (h d) -> p h d", h=NHL),
                        bv_sb.rearrange("p (h d) -> p h d", h=NHL),
                        mybir.AluOpType.add,
                    )

        # ---------------- stage 2: causal attention ----------------
        # wp is only needed by stage 3 — load it here so it doesn't
        # compete with the stage-1 weight/x streams
        nc.sync.dma_start(wp_sb, wp[:])
        with (
            tc.tile_pool(name="s2att", bufs=4) as s2att,
            tc.tile_pool(name="s2n", bufs=3) as s2n,
            tc.tile_pool(name="qkps", bufs=2, space="PSUM") as qkps,
            tc.tile_pool(name="pvps", bufs=4, space="PSUM") as pvps,
        ):
            for a in range(NPAIR):
                for it in range(NIT):
                    i0 = it * TQ
                    njt = (i0 + TQ) // TJ
                    pv = [
                        pvps.tile([P, TQ], F32, tag="pv", name=f"pv0_{a}_{it}"),
                        pvps.tile([P, TQ], F32, tag="pv", name=f"pv1_{a}_{it}"),
                    ]
                    for jt in range(njt):
                        j0 = jt * TJ
                        d = j0 - i0
                        istart = max(d, 0)
                        nn = TQ - istart
                        # one 2-bank psum tile holds both heads' S^T blocks;
                        # a single fused exp call halves ACT instruction count
                        qk = qkps.tile([P, 2, TQ], F32, tag="qk")
                        for e in (0, 1):
                            nc.tensor.matmul(
                                qk[:, e, istart:TQ],
                                k_sb[64 * e : 64 * e + 64, a, j0 : j0 + TJ],
                                q_sb[64 * e : 64 * e + 64, a, i0 + istart : i0 + TQ],
                                start=True,
                                stop=(d < 0),
                                tile_position=(64 * e, 0),
                            )
                            if d >= 0:
                                nc.tensor.matmul(
                                    qk[:, e, istart : istart + TJ],
                                    idn_sb,
                                    tri_sb,
                                    start=False,
                                    stop=True,
                                    tile_position=(0, 0),
                                )
                        att = s2att.tile([P, 2, TQ], F32R, tag="att")
                        nc.scalar.activation(
                            att[:, :, 0:nn], qk[:, :, istart:TQ], AF.Exp
                        )
                        for e in (0, 1):
                            nc.tensor.matmul(
                                pv[e][0 : HD + 1, istart:TQ],
                                v_sb[:, jt, 2 * a + e, :],
                                att[:, e, 0:nn],
                                start=(jt == 0),
                                stop=(jt == njt - 1),
                            )
                    for e in (0, 1):
                        rt = s2n.tile([P, TQ], F32, tag="rt")
                        nc.vector.reciprocal(rt[HD : HD + 1, :], pv[e][HD : HD + 1, :])
                        rb = s2n.tile([P, TQ], F32, tag="rb")
                        nc.sync.dma_start(rb[0:1, :], rt[HD : HD + 1, :])
                        nc.gpsimd.partition_broadcast(rb[0:HD, :], rb[0:1, :])
                        if e == 0:
                            nc.vector.tensor_mul(
                                q_sb[0:HD, a, i0 : i0 + TQ], pv[e][0:HD, :], rb[0:HD, :]
                            )
                        else:
                            yt = s2n.tile([P, TQ], F32R, tag="yt")
                            nc.vector.tensor_mul(yt[0:HD, :], pv[e][0:HD, :], rb[0:HD, :])
                            nc.sync.dma_start(
                                q_sb[64:128, a, i0 : i0 + TQ], yt[0:HD, :]
                            )


        # ---------------- stage 3: output projection ----------------
        # normalized y^T was written into q_sb (reuse; q no longer needed)
        with (
            tc.tile_pool(name="s3o", bufs=4) as s3o,
            tc.tile_pool(name="s3ps", bufs=4, space="PSUM") as s3ps,
        ):
            for tt in range(NTS):
                for ot in range(2):
                    ps = s3ps.tile([P, 512], F32, tag="ops")
                    for a in range(NPAIR):
                        nc.tensor.matmul(
                            ps,
                            q_sb[:, a, tt * P : (tt + 1) * P],
                            wp_sb[:, a, ot * 512 : (ot + 1) * 512],
                            start=(a == 0),
                            stop=(a == NPAIR - 1),
                        )
                    ot_sb = s3o.tile([P, 512], F32, tag="osb")
                    nc.vector.tensor_copy(ot_sb, ps)
                    nc.sync.dma_start(
                        out[tt * P : (tt + 1) * P, ot * 512 : (ot + 1) * 512], ot_sb
                    )

    nc.compile()
    return nc


_NC_CACHE = None


def _get_nc():
    global _NC_CACHE
    if _NC_CACHE is None:
        _NC_CACHE = build_kernel()
    return _NC_CACHE


def _shard_inputs(x, w_qkv, b_qkv, w_proj):
    """Build the 8 per-core input maps. Core id = 2*batch + head_group."""
    tri_np = np.where(
        np.arange(P)[None, :] >= np.arange(P)[:, None], 0.0, NEG
    ).astype(ml_dtypes.bfloat16)
    idn_np = np.eye(P, dtype=ml_dtypes.bfloat16)

    in_maps = []
    for b in range(B):
        xt = np.ascontiguousarray(x[b].T)  # [C, T]
        for g in range(2):
            s = slice(g * 512, (g + 1) * 512)
            wqk_full = np.concatenate(
                [w_qkv[0:1024][s] / 8.0, w_qkv[1024:2048][s]], axis=0
            )  # [1024 f, 1024 c]
            wqk_arr = np.ascontiguousarray(
                wqk_full.T.reshape(NCT, P, 1024).transpose(1, 0, 2)
            )
            bqk_full = np.concatenate([b_qkv[0:1024][s] / 8.0, b_qkv[1024:2048][s]])
            bqk_arr = np.ascontiguousarray(bqk_full.reshape(8, P).T)
            wv_rows = w_qkv[2048:3072][s]  # [512 f, 1024 c]
            wv_arr = np.ascontiguousarray(
                wv_rows.T.reshape(NCT, P, 512).transpose(1, 0, 2)
            )
            bv_arr = np.ascontiguousarray(b_qkv[2048:3072][s][None, :])
            wp_rhs = w_proj[:, s].T  # [512 hd, 1024 o]
            wp_arr = np.ascontiguousarray(
                wp_rhs.reshape(NPAIR, P, 1024).transpose(1, 0, 2)
            )
            in_maps.append(
                {
                    "xt": xt,
                    "wqk": wqk_arr.astype(np.float32),
                    "bqk": bqk_arr.astype(np.float32),
                    "wv": wv_arr.astype(np.float32),
                    "bv": bv_arr.astype(np.float32),
                    "wp": wp_arr.astype(np.float32),
                    "tri": tri_np,
                    "idn": idn_np,
                }
            )
    return in_maps


def kernel(x, w_qkv, b_qkv, w_proj, b_proj, _trace=False, _trace_kwargs=None):
    x = np.asarray(x, dtype=np.float32)
    w_qkv = np.asarray(w_qkv, dtype=np.float32)
    b_qkv = np.asarray(b_qkv, dtype=np.float32)
    w_proj = np.asarray(w_proj, dtype=np.float32)
    b_proj = np.asarray(b_proj, dtype=np.float32)

    nc = _get_nc()
    in_maps = _shard_inputs(x, w_qkv, b_qkv, w_proj)
    res = run_bass_kernel_spmd(
        nc, in_maps, core_ids=list(range(8)), trace=_trace,
        **(_trace_kwargs or {}),
    )
    out = np.empty((B, T, C), np.float32)
    for b in range(B):
        out[b] = res.results[2 * b]["out"] + res.results[2 * b + 1]["out"] + b_proj
    if _trace:
        return out, res
    return out
